# revision 2
# baseline (speedup 1.0000x reference)
"""Trainium2 Bass kernel for the deterministic legality module.

Computes, for each board b, filter f and top-left placement (i,j):
    legal[b,f,i,j] = 1.0 iff every occupied cell of filter f, placed at
    (i,j), lands in-bounds on a free cell of board b (and f is non-empty).

Reformulated as one matmul per output tile:
    out[b, f*81+ij] = relu( sum_k boardX[b,k] * M[k, f*81+ij] )
where rows 0..80 of M hold filter f placed at ij (zero out of bounds) and
rows 81,82 hold the two integer halves of thr[f] = 1-area (or -1 for empty
filters); boardX appends two ones-columns to the board.  corr <= area
always, so relu(corr + thr) is exactly the 0/1 legality.

M is input-dependent (filters) but built on the HOST in numpy and uploaded
in fp8e4 (all entries are 0/1 or integers in [-12,0] -- exact in e4m3).
That removes the on-device M-build entirely.  The main matmul runs in fp8
(same PE rate as bf16, quarter the HBM traffic), the relu+downcast pass is
split DVE/ACT 4:5 over four-bank PSUM tiles, and the output is written to
HBM as fp8 (0/1 exact) and upcast to f32 on the host -- output DMA is the
baseline's bottleneck (43.8 MB f32/core, ~122 us) and shrinks 4x.

Sharding: pure data parallelism, batch 4096 -> 512 per core on 8 cores.
"""

import numpy as np
import ml_dtypes

N_CORES = 8
BATCH = 4096
BPC = BATCH // N_CORES  # 512 boards per core
NPOS = 81               # 9x9 board cells / placements
NF = 264                # filters
NCOL = NF * NPOS        # 21384 output columns per board
K = NPOS + 2            # contraction: 81 board cells + 2 threshold rows

COL_TILE = 512          # one PSUM bank of f32
COL_GROUP = 2048        # 4 banks per PSUM tile / relu op / output DMA
N_SLABS = 8             # M upload slabs (21384 / 8 = 2673 cols each)
# DVE:ACT relu split ~ 0.96:1.2 GHz -> 4:5 pattern of period 9
DVE_SLOTS = (0, 2, 4, 6)


def _build_m(filters: np.ndarray, areas: np.ndarray) -> np.ndarray:
    """M [83, 21384] fp8e4: placed-filter geometry + threshold rows."""
    F = np.asarray(filters, dtype=np.float32).reshape(NF, 5, 5)
    M = np.zeros((K, NF, NPOS), dtype=np.float32)
    for i in range(9):
        h = min(5, 9 - i)
        for j in range(9):
            w = min(5, 9 - j)
            blk = np.zeros((NF, 9, 9), dtype=np.float32)
            blk[:, i:i + h, j:j + w] = F[:, :h, :w]
            M[:NPOS, :, i * 9 + j] = blk.reshape(NF, NPOS).T
    ar = np.asarray(areas, dtype=np.float32).reshape(NF)
    thr = np.where(ar > 0.5, 1.0 - ar, -1.0)
    lo = np.floor(thr / 2.0)
    M[NPOS, :, :] = lo[:, None]          # floor(thr/2)   in [-12, 0]
    M[NPOS + 1, :, :] = (thr - lo)[:, None]  # ceil(thr/2) in [-12, 0]
    return M.reshape(K, NCOL).astype(ml_dtypes.float8_e4m3)


def _build_module():
    import concourse.bass as bass
    import concourse.mybir as mybir
    import concourse.tile as tile
    from concourse.masks import make_identity

    f32 = mybir.dt.float32
    fp8 = mybir.dt.float8e4
    Relu = mybir.ActivationFunctionType.Relu

    nc = bass.Bass("TRN2", target_bir_lowering=False, debug=False,
                   num_devices=N_CORES)

    board_d = nc.dram_tensor("board", [BPC, NPOS], f32, kind="ExternalInput")
    m_d = nc.dram_tensor("mmat", [K, NCOL], fp8, kind="ExternalInput")
    out_d = nc.dram_tensor("out", [BPC, NCOL], fp8, kind="ExternalOutput")

    with tile.TileContext(nc) as tc:
        with tc.tile_pool(name="const", bufs=1) as cpool:
            ident = cpool.tile([128, 128], f32)
            make_identity(nc, ident[:])

            boardT = cpool.tile([K, BPC], fp8)    # [81 cells + 2 ones rows]
            msb = cpool.tile([K, NCOL], fp8)

            # M streams in as 8 column slabs; main-loop matmuls over a
            # column range depend only on the slabs covering it.
            slab = NCOL // N_SLABS
            for s in range(N_SLABS):
                nc.sync.dma_start(msb[:, s * slab:(s + 1) * slab],
                                  m_d[:, s * slab:(s + 1) * slab])

            # board (512,81) f32 -> boardT[0:81,:] fp8 (transposed), with
            # two ones-columns appended so the transpose also produces the
            # ones rows at partitions 81,82.  One merged DMA.
            with (
                tc.tile_pool(name="prep", bufs=1) as prep,
                tc.tile_pool(name="psA", bufs=2, space="PSUM") as psA,
            ):
                btile = prep.tile([128, 4 * K], f32, tag="bload")
                bt3 = btile[:].rearrange("p (c y) -> p c y", c=4)
                nc.gpsimd.memset(bt3[:, :, NPOS:K], 1.0)
                bview = board_d[:].rearrange("(c p) x -> p c x", p=128)
                nc.sync.dma_start(bt3[:, :, 0:NPOS], bview)
                for kb in range(BPC // 128):
                    bps = psA.tile([K, 128], f32, tag="btp")
                    nc.tensor.transpose(bps[:], btile[:, kb * K:(kb + 1) * K],
                                        ident[:])
                    nc.vector.tensor_scalar_add(
                        boardT[:, kb * 128:(kb + 1) * 128], bps[:], 0.0)

            # ---- main loop: matmul + relu/downcast + store ------------
            with (
                tc.tile_pool(name="psM", bufs=2, space="PSUM") as psM,
                tc.tile_pool(name="ostage", bufs=4) as ostage,
            ):
                grp = 0
                for kb in range(BPC // 128):
                    lhsT = boardT[:, kb * 128:(kb + 1) * 128]
                    for g0 in range(0, NCOL, COL_GROUP):
                        gw = min(COL_GROUP, NCOL - g0)
                        pt = psM.tile([128, COL_GROUP], f32, tag="mm")
                        for q in range(0, gw, COL_TILE):
                            w = min(COL_TILE, gw - q)
                            nc.tensor.matmul(pt[:, q:q + w], lhsT,
                                             msb[:, g0 + q:g0 + q + w],
                                             start=True, stop=True)
                        ot = ostage.tile([128, COL_GROUP], fp8, tag="ot")
                        if grp % 9 in DVE_SLOTS:
                            nc.vector.tensor_scalar_max(
                                ot[:, :gw], pt[:, :gw], 0.0)
                        else:
                            nc.scalar.activation(ot[:, :gw], pt[:, :gw], Relu)
                        grp += 1
                        nc.sync.dma_start(
                            out_d[kb * 128:(kb + 1) * 128, g0:g0 + gw],
                            ot[:, :gw])
    return nc


def _legalize_multiwait(nc):
    """Split multi-wait instructions for this walrus build.

    The TPB instruction encodings carry exactly one semaphore wait, and
    the walrus codegen here refuses instructions with more ("Too many
    sync wait commands").  Hoist all but one wait onto EventSemaphore
    carrier instructions placed immediately before, on the same engine —
    the sequencer blocks on each carrier first, which is semantically
    identical.
    """
    import concourse.mybir as mybir

    for func in nc.m.functions:
        for blk in func.blocks:
            out = []
            changed = False
            for inst in blk.instructions:
                si = inst.sync_info
                waits = list(si.on_wait) if si is not None and si.on_wait else []
                if len(waits) > 1:
                    for j, w in enumerate(waits[:-1]):
                        carrier = mybir.InstEventSemaphore(
                            name=f"{inst.name}-xw{j}",
                            engine=inst.engine,
                            ins=[], outs=[],
                            sync_info=mybir.SyncInfo(on_wait=[w],
                                                     on_update=[]),
                        )
                        nc.register_instruction(carrier)
                        out.append(carrier)
                    inst.sync_info = mybir.SyncInfo(
                        on_wait=[waits[-1]],
                        on_update=list(si.on_update) if si.on_update else [])
                    changed = True
                out.append(inst)
            if changed:
                blk.instructions = out


_MODULE = None


def _get_module():
    global _MODULE
    if _MODULE is None:
        _MODULE = _build_module()
        _legalize_multiwait(_MODULE)
    return _MODULE


def run(board_free, filters, areas, trace=False, **spmd_kwargs):
    from concourse.bass_utils import run_bass_kernel_spmd

    board = np.ascontiguousarray(
        np.asarray(board_free, dtype=np.float32).reshape(N_CORES, BPC, NPOS))
    mmat = _build_m(filters, areas)

    in_maps = [
        {"board": board[c], "mmat": mmat}
        for c in range(N_CORES)
    ]
    nc = _get_module()
    res = run_bass_kernel_spmd(nc, in_maps, core_ids=list(range(N_CORES)),
                               trace=trace, **spmd_kwargs)
    out = np.concatenate(
        [np.asarray(r["out"]).astype(np.float32) for r in res.results], axis=0)
    out = out.reshape(BATCH, NF, 9, 9)
    return out, res


def kernel(board_free, filters, areas):
    out, _ = run(board_free, filters, areas)
    return out


# revision 6
# speedup vs baseline: 1.5903x; 1.5903x over previous
"""Trainium2 Bass kernel for the deterministic legality module.

Computes, for each board b, filter f and top-left placement (i,j):
    legal[b,f,i,j] = 1.0 iff every occupied cell of filter f, placed at
    (i,j), lands in-bounds on a free cell of board b (and f is non-empty).

Reformulated as one matmul per output tile:
    out[b, f*81+ij] = relu( sum_k boardX[b,k] * M[k, f*81+ij] )
where rows 0..80 of M hold filter f placed at ij (zero out of bounds) and
rows 81,82 hold the two integer halves of thr[f] = 1-area (or -1 for empty
filters); boardX appends two ones-columns to the board.  corr <= area
always, so relu(corr + thr) is exactly the 0/1 legality.

M is input-dependent (filters) but built on the HOST in numpy and uploaded
in fp8e4 (all entries are 0/1 or integers in [-12,0] -- exact in e4m3).
That removes the on-device M-build entirely.  The main matmul runs in fp8
(same PE rate as bf16, quarter the HBM traffic), the relu+downcast pass is
split DVE/ACT 4:5 over four-bank PSUM tiles, and the output is written to
HBM as fp8 (0/1 exact) and upcast to f32 on the host -- output DMA is the
baseline's bottleneck (43.8 MB f32/core, ~122 us) and shrinks 4x.

Sharding: pure data parallelism, batch 4096 -> 512 per core on 8 cores.
"""

import numpy as np
import ml_dtypes

N_CORES = 8
BATCH = 4096
BPC = BATCH // N_CORES  # 512 boards per core
NPOS = 81               # 9x9 board cells / placements
NF = 264                # filters
NCOL = NF * NPOS        # 21384 output columns per board
K = NPOS + 2            # contraction: 81 board cells + 2 threshold rows

COL_TILE = 512          # one PSUM bank of f32
COL_GROUP = 2048        # 4 banks per PSUM tile / relu op / output DMA
N_SLABS = 8             # M upload slabs (21384 / 8 = 2673 cols each)
# DVE:ACT relu split ~ 0.96:1.2 GHz -> 4:5 pattern of period 9
DVE_SLOTS = (0, 2, 4, 6)


KPAD = 128              # M uploaded padded to 128 partitions: DMA engines
                        # have fixed partition affinity, so a 128-partition
                        # transfer fans out across all 16 engines while an
                        # 83-partition one serializes (measured: 58us vs 8us)


def _build_m(filters: np.ndarray, areas: np.ndarray) -> np.ndarray:
    """M [128, 21384] fp8e4: placed-filter geometry + threshold rows + pad."""
    F = np.asarray(filters, dtype=np.float32).reshape(NF, 5, 5)
    M = np.zeros((KPAD, NF, NPOS), dtype=np.float32)
    for i in range(9):
        h = min(5, 9 - i)
        for j in range(9):
            w = min(5, 9 - j)
            blk = np.zeros((NF, 9, 9), dtype=np.float32)
            blk[:, i:i + h, j:j + w] = F[:, :h, :w]
            M[:NPOS, :, i * 9 + j] = blk.reshape(NF, NPOS).T
    ar = np.asarray(areas, dtype=np.float32).reshape(NF)
    thr = np.where(ar > 0.5, 1.0 - ar, -1.0)
    lo = np.floor(thr / 2.0)
    M[NPOS, :, :] = lo[:, None]          # floor(thr/2)   in [-12, 0]
    M[NPOS + 1, :, :] = (thr - lo)[:, None]  # ceil(thr/2) in [-12, 0]
    return M.reshape(KPAD, NCOL).astype(ml_dtypes.float8_e4m3)


def _build_module():
    import concourse.bass as bass
    import concourse.mybir as mybir
    import concourse.tile as tile
    from concourse.masks import make_identity

    f32 = mybir.dt.float32
    fp8 = mybir.dt.float8e4
    Relu = mybir.ActivationFunctionType.Relu

    nc = bass.Bass("TRN2", target_bir_lowering=False, debug=False,
                   num_devices=N_CORES)

    board_d = nc.dram_tensor("board", [BPC, NPOS], f32, kind="ExternalInput")
    m_d = nc.dram_tensor("mmat", [KPAD, NCOL], fp8, kind="ExternalInput")
    out_d = nc.dram_tensor("out", [BPC, NCOL], fp8, kind="ExternalOutput")

    with tile.TileContext(nc) as tc:
        with tc.tile_pool(name="const", bufs=1) as cpool:
            ident = cpool.tile([128, 128], f32)
            make_identity(nc, ident[:])

            boardT = cpool.tile([K, BPC], fp8)    # [81 cells + 2 ones rows]
            msb = cpool.tile([KPAD, NCOL], fp8)

            # board (512,81) f32 -> boardT[0:81,:] fp8 (transposed), with
            # two ones-columns appended so the transpose also produces the
            # ones rows at partitions 81,82.  One merged DMA.
            # Inputs go on the ACT hwdge ring (board first), outputs on the
            # SP ring -- the rings are FIFO per issuing engine, so this
            # keeps output stores from queueing behind the M upload.
            with (
                tc.tile_pool(name="prep", bufs=1) as prep,
                tc.tile_pool(name="psA", bufs=2, space="PSUM") as psA,
            ):
                btile = prep.tile([128, 4 * K], f32, tag="bload")
                bt3 = btile[:].rearrange("p (c y) -> p c y", c=4)
                nc.gpsimd.memset(bt3[:, :, NPOS:K], 1.0)
                bview = board_d[:].rearrange("(c p) x -> p c x", p=128)
                nc.scalar.dma_start(bt3[:, :, 0:NPOS], bview)

                # M streams in as 8 column slabs; main-loop matmuls over a
                # column range depend only on the slabs covering it.
                slab = NCOL // N_SLABS
                for s in range(N_SLABS):
                    nc.scalar.dma_start(msb[:, s * slab:(s + 1) * slab],
                                        m_d[:, s * slab:(s + 1) * slab])
                for kb in range(BPC // 128):
                    bps = psA.tile([K, 128], f32, tag="btp")
                    nc.tensor.transpose(bps[:], btile[:, kb * K:(kb + 1) * K],
                                        ident[:])
                    nc.vector.tensor_scalar_add(
                        boardT[:, kb * 128:(kb + 1) * 128], bps[:], 0.0)

            # ---- main loop: matmul + relu/downcast + store ------------
            with (
                tc.tile_pool(name="psM", bufs=2, space="PSUM") as psM,
                tc.tile_pool(name="ostage", bufs=4) as ostage,
            ):
                grp = 0
                for kb in range(BPC // 128):
                    lhsT = boardT[:, kb * 128:(kb + 1) * 128]
                    for g0 in range(0, NCOL, COL_GROUP):
                        gw = min(COL_GROUP, NCOL - g0)
                        pt = psM.tile([128, COL_GROUP], f32, tag="mm")
                        for q in range(0, gw, COL_TILE):
                            w = min(COL_TILE, gw - q)
                            nc.tensor.matmul(pt[:, q:q + w], lhsT,
                                             msb[0:K, g0 + q:g0 + q + w],
                                             start=True, stop=True)
                        ot = ostage.tile([128, COL_GROUP], fp8, tag="ot")
                        if grp % 9 in DVE_SLOTS:
                            nc.vector.tensor_scalar_max(
                                ot[:, :gw], pt[:, :gw], 0.0)
                        else:
                            nc.scalar.activation(ot[:, :gw], pt[:, :gw], Relu)
                        grp += 1
                        nc.sync.dma_start(
                            out_d[kb * 128:(kb + 1) * 128, g0:g0 + gw],
                            ot[:, :gw])
    return nc


def _legalize_multiwait(nc):
    """Split multi-wait instructions for this walrus build.

    The TPB instruction encodings carry exactly one semaphore wait, and
    the walrus codegen here refuses instructions with more ("Too many
    sync wait commands").  Hoist all but one wait onto EventSemaphore
    carrier instructions placed immediately before, on the same engine —
    the sequencer blocks on each carrier first, which is semantically
    identical.
    """
    import concourse.mybir as mybir

    for func in nc.m.functions:
        for blk in func.blocks:
            out = []
            changed = False
            for inst in blk.instructions:
                si = inst.sync_info
                waits = list(si.on_wait) if si is not None and si.on_wait else []
                if len(waits) > 1:
                    for j, w in enumerate(waits[:-1]):
                        carrier = mybir.InstEventSemaphore(
                            name=f"{inst.name}-xw{j}",
                            engine=inst.engine,
                            ins=[], outs=[],
                            sync_info=mybir.SyncInfo(on_wait=[w],
                                                     on_update=[]),
                        )
                        nc.register_instruction(carrier)
                        out.append(carrier)
                    inst.sync_info = mybir.SyncInfo(
                        on_wait=[waits[-1]],
                        on_update=list(si.on_update) if si.on_update else [])
                    changed = True
                out.append(inst)
            if changed:
                blk.instructions = out


_MODULE = None


def _get_module():
    global _MODULE
    if _MODULE is None:
        _MODULE = _build_module()
        _legalize_multiwait(_MODULE)
    return _MODULE


def run(board_free, filters, areas, trace=False, **spmd_kwargs):
    from concourse.bass_utils import run_bass_kernel_spmd

    board = np.ascontiguousarray(
        np.asarray(board_free, dtype=np.float32).reshape(N_CORES, BPC, NPOS))
    mmat = _build_m(filters, areas)

    in_maps = [
        {"board": board[c], "mmat": mmat}
        for c in range(N_CORES)
    ]
    nc = _get_module()
    res = run_bass_kernel_spmd(nc, in_maps, core_ids=list(range(N_CORES)),
                               trace=trace, **spmd_kwargs)
    out = np.concatenate(
        [np.asarray(r["out"]).astype(np.float32) for r in res.results], axis=0)
    out = out.reshape(BATCH, NF, 9, 9)
    return out, res


def kernel(board_free, filters, areas):
    out, _ = run(board_free, filters, areas)
    return out


# revision 7
# speedup vs baseline: 2.0400x; 1.2828x over previous
"""Trainium2 Bass kernel for the deterministic legality module.

Computes, for each board b, filter f and top-left placement (i,j):
    legal[b,f,i,j] = 1.0 iff every occupied cell of filter f, placed at
    (i,j), lands in-bounds on a free cell of board b (and f is non-empty).

Reformulated as one matmul per output tile:
    out[b, f*81+ij] = relu( sum_k boardX[b,k] * M[k, f*81+ij] )
where rows 0..80 of M hold filter f placed at ij (zero out of bounds) and
rows 81,82 hold the two integer halves of thr[f] = 1-area (or -1 for empty
filters); boardX appends two ones-columns to the board.  corr <= area
always, so relu(corr + thr) is exactly the 0/1 legality.

Both M and the transposed board are built on the HOST in numpy and
uploaded in fp8e4 (entries are 0/1 or integers in [-12,0] -- exact in
e4m3), padded to 128 partitions: the DMA engines have fixed partition
affinity, so 128-partition transfers fan out across all 16 engines while
narrow ones serialize.  The device is then a single pipeline:
  matmul (fp8, PSUM f32) -> relu+fp8 downcast (DVE/ACT, 5:6 split)
  -> HBM store (fp8, upcast to f32 on host).
A few fp32 dummy matmuls at the head keep the PE busy ~7us so the HAM
clock gate lifts 1.2 -> 2.4 GHz before the main loop.

Sharding: pure data parallelism, batch 4096 -> 512 per core on 8 cores.
"""

import numpy as np
import ml_dtypes

N_CORES = 8
BATCH = 4096
BPC = BATCH // N_CORES  # 512 boards per core
NPOS = 81               # 9x9 board cells / placements
NF = 264                # filters
NCOL = NF * NPOS        # 21384 output columns per board
K = NPOS + 2            # contraction: 81 board cells + 2 threshold rows
KPAD = 128              # uploads padded to 128 partitions for DMA fan-out

COL_TILE = 512          # one PSUM bank of f32
GRP = 1024              # 2 banks per PSUM ring slot / relu op
DMA_GRP = 2048          # output staging tile / store DMA
N_SLABS = 8             # M upload slabs (21384 / 8 = 2673 cols each)
WARMUP_MM = 5           # fp32 N=256 dummy matmuls (~4.3us) for the HAM gate
# DVE:ACT relu split ~ 0.96:1.2 GHz incl. overheads -> 5:6 pattern
DVE_SLOTS = (0, 2, 4, 6, 8)
PERIOD = 11


def _build_m(filters: np.ndarray, areas: np.ndarray) -> np.ndarray:
    """M [128, 21384] fp8e4: placed-filter geometry + threshold rows + pad."""
    F = np.asarray(filters, dtype=np.float32).reshape(NF, 5, 5)
    M = np.zeros((KPAD, NF, NPOS), dtype=np.float32)
    for i in range(9):
        h = min(5, 9 - i)
        for j in range(9):
            w = min(5, 9 - j)
            blk = np.zeros((NF, 9, 9), dtype=np.float32)
            blk[:, i:i + h, j:j + w] = F[:, :h, :w]
            M[:NPOS, :, i * 9 + j] = blk.reshape(NF, NPOS).T
    ar = np.asarray(areas, dtype=np.float32).reshape(NF)
    thr = np.where(ar > 0.5, 1.0 - ar, -1.0)
    lo = np.floor(thr / 2.0)
    M[NPOS, :, :] = lo[:, None]          # floor(thr/2)   in [-12, 0]
    M[NPOS + 1, :, :] = (thr - lo)[:, None]  # ceil(thr/2) in [-12, 0]
    return M.reshape(KPAD, NCOL).astype(ml_dtypes.float8_e4m3)


def _build_boardt(board_free: np.ndarray) -> np.ndarray:
    """boardT [cores, 128, 512] fp8e4: transposed boards + ones rows + pad."""
    b = np.asarray(board_free, dtype=np.float32).reshape(N_CORES, BPC, NPOS)
    bt = np.zeros((N_CORES, KPAD, BPC), dtype=np.float32)
    bt[:, :NPOS, :] = b.transpose(0, 2, 1)
    bt[:, NPOS:K, :] = 1.0
    return bt.astype(ml_dtypes.float8_e4m3)


def _build_module():
    import concourse.bass as bass
    import concourse.mybir as mybir
    import concourse.tile as tile

    f32 = mybir.dt.float32
    fp8 = mybir.dt.float8e4
    Relu = mybir.ActivationFunctionType.Relu

    nc = bass.Bass("TRN2", target_bir_lowering=False, debug=False,
                   num_devices=N_CORES)

    boardt_d = nc.dram_tensor("boardt", [KPAD, BPC], fp8, kind="ExternalInput")
    m_d = nc.dram_tensor("mmat", [KPAD, NCOL], fp8, kind="ExternalInput")
    out_d = nc.dram_tensor("out", [BPC, NCOL], fp8, kind="ExternalOutput")

    with tile.TileContext(nc) as tc:
        with tc.tile_pool(name="const", bufs=1) as cpool:
            boardT = cpool.tile([KPAD, BPC], fp8)
            msb = cpool.tile([KPAD, NCOL], fp8)

            # Inputs on the ACT hwdge ring (board first), outputs on the SP
            # ring -- rings are FIFO per issuing engine, so output stores
            # never queue behind the M upload.  M streams in as 8 column
            # slabs; matmuls over a column range depend only on its slabs.
            nc.scalar.dma_start(boardT[:], boardt_d[:])
            slab = NCOL // N_SLABS
            for s in range(N_SLABS):
                nc.scalar.dma_start(msb[:, s * slab:(s + 1) * slab],
                                    m_d[:, s * slab:(s + 1) * slab])

            # ---- HAM warm-up: fp32 dummy matmuls while inputs upload ----
            # The PE clock gate sits at 1.2 GHz until the activity monitor
            # sees a ~3.4us window of sustained matmul activity; these
            # bridge into the main loop so it runs at 2.4 GHz.
            with (
                tc.tile_pool(name="wprep", bufs=1) as wprep,
                tc.tile_pool(name="psW", bufs=1, space="PSUM") as psW,
            ):
                dummy = wprep.tile([128, 256], f32, tag="wsrc")
                nc.gpsimd.memset(dummy[:], 0.0)
                wps = psW.tile([128, 256], f32, tag="warm")
                for _ in range(WARMUP_MM):
                    nc.tensor.matmul(wps[:], dummy[:, 0:128], dummy[:],
                                     start=True, stop=True)
                wrd = wprep.tile([32, 1], f32, tag="wrd")
                nc.vector.tensor_scalar_add(wrd[:], wps[0:32, 0:1], 0.0)

            # ---- main loop: matmul + relu/downcast + store ------------
            with (
                tc.tile_pool(name="psM", bufs=4, space="PSUM") as psM,
                tc.tile_pool(name="ostage", bufs=4) as ostage,
            ):
                grp = 0
                for kb in range(BPC // 128):
                    lhsT = boardT[0:K, kb * 128:(kb + 1) * 128]
                    for g0 in range(0, NCOL, DMA_GRP):
                        dw = min(DMA_GRP, NCOL - g0)
                        ot = ostage.tile([128, DMA_GRP], fp8, tag="ot")
                        for h0 in range(0, dw, GRP):
                            hw = min(GRP, dw - h0)
                            pt = psM.tile([128, GRP], f32, tag="mm")
                            for q in range(0, hw, COL_TILE):
                                w = min(COL_TILE, hw - q)
                                c = g0 + h0 + q
                                nc.tensor.matmul(pt[:, q:q + w], lhsT,
                                                 msb[0:K, c:c + w],
                                                 start=True, stop=True)
                            if grp % PERIOD in DVE_SLOTS:
                                nc.vector.tensor_scalar_max(
                                    ot[:, h0:h0 + hw], pt[:, :hw], 0.0)
                            else:
                                nc.scalar.activation(ot[:, h0:h0 + hw],
                                                     pt[:, :hw], Relu)
                            grp += 1
                        nc.sync.dma_start(
                            out_d[kb * 128:(kb + 1) * 128, g0:g0 + dw],
                            ot[:, :dw])
    return nc


def _legalize_multiwait(nc):
    """Split multi-wait instructions for this walrus build.

    The TPB instruction encodings carry exactly one semaphore wait, and
    the walrus codegen here refuses instructions with more ("Too many
    sync wait commands").  Hoist all but one wait onto EventSemaphore
    carrier instructions placed immediately before, on the same engine —
    the sequencer blocks on each carrier first, which is semantically
    identical.
    """
    import concourse.mybir as mybir

    for func in nc.m.functions:
        for blk in func.blocks:
            out = []
            changed = False
            for inst in blk.instructions:
                si = inst.sync_info
                waits = list(si.on_wait) if si is not None and si.on_wait else []
                if len(waits) > 1:
                    for j, w in enumerate(waits[:-1]):
                        carrier = mybir.InstEventSemaphore(
                            name=f"{inst.name}-xw{j}",
                            engine=inst.engine,
                            ins=[], outs=[],
                            sync_info=mybir.SyncInfo(on_wait=[w],
                                                     on_update=[]),
                        )
                        nc.register_instruction(carrier)
                        out.append(carrier)
                    inst.sync_info = mybir.SyncInfo(
                        on_wait=[waits[-1]],
                        on_update=list(si.on_update) if si.on_update else [])
                    changed = True
                out.append(inst)
            if changed:
                blk.instructions = out


_MODULE = None


def _get_module():
    global _MODULE
    if _MODULE is None:
        _MODULE = _build_module()
        _legalize_multiwait(_MODULE)
    return _MODULE


def run(board_free, filters, areas, trace=False, **spmd_kwargs):
    from concourse.bass_utils import run_bass_kernel_spmd

    boardt = _build_boardt(board_free)
    mmat = _build_m(filters, areas)

    in_maps = [
        {"boardt": boardt[c], "mmat": mmat}
        for c in range(N_CORES)
    ]
    nc = _get_module()
    res = run_bass_kernel_spmd(nc, in_maps, core_ids=list(range(N_CORES)),
                               trace=trace, **spmd_kwargs)
    out = np.concatenate(
        [np.asarray(r["out"]).astype(np.float32) for r in res.results], axis=0)
    out = out.reshape(BATCH, NF, 9, 9)
    return out, res


def kernel(board_free, filters, areas):
    out, _ = run(board_free, filters, areas)
    return out


# revision 9
# speedup vs baseline: 2.1229x; 1.0406x over previous
"""Trainium2 Bass kernel for the deterministic legality module.

Computes, for each board b, filter f and top-left placement (i,j):
    legal[b,f,i,j] = 1.0 iff every occupied cell of filter f, placed at
    (i,j), lands in-bounds on a free cell of board b (and f is non-empty).

Reformulated as one matmul per output tile:
    out[b, f*81+ij] = relu( sum_k boardX[b,k] * M[k, f*81+ij] )
where rows 0..80 of M hold filter f placed at ij (zero out of bounds) and
rows 81,82 hold the two integer halves of thr[f] = 1-area (or -1 for empty
filters); boardX appends two ones-columns to the board.  corr <= area
always, so relu(corr + thr) is exactly the 0/1 legality.

Both M and the transposed board are built on the HOST in numpy and
uploaded in fp8e4 (entries are 0/1 or integers in [-12,0] -- exact in
e4m3), padded to 128 partitions: the DMA engines have fixed partition
affinity, so 128-partition transfers fan out across all 16 engines while
narrow ones serialize.  The device is then a single pipeline:
  matmul (fp8, PSUM f32) -> relu+fp8 downcast (DVE/ACT, 5:6 split)
  -> HBM store (fp8, upcast to f32 on host).
A few fp32 dummy matmuls at the head keep the PE busy ~7us so the HAM
clock gate lifts 1.2 -> 2.4 GHz before the main loop.

Sharding: pure data parallelism, batch 4096 -> 512 per core on 8 cores.
"""

import numpy as np
import ml_dtypes

N_CORES = 8
BATCH = 4096
BPC = BATCH // N_CORES  # 512 boards per core
NPOS = 81               # 9x9 board cells / placements
NF = 264                # filters
NCOL = NF * NPOS        # 21384 output columns per board
K = NPOS + 2            # contraction: 81 board cells + 2 threshold rows
KPAD = 128              # uploads padded to 128 partitions for DMA fan-out

COL_TILE = 512          # one PSUM bank of f32
GRP = 1024              # 2 banks per PSUM ring slot / relu op
DMA_GRP = 2048          # output staging tile / store DMA
N_SLABS = 8             # M upload slabs (21384 / 8 = 2673 cols each)
WARMUP_MM = 6           # fp32 N=256 dummy matmuls (~5us) to lift the HAM gate
FILLER_MM = 2           # fp8 N=128 dummy matmuls per group to HOLD it: the
                        # gate re-throttles to 1.2 GHz unless PE activity is
                        # near-contiguous, and warm PE (430ns/group) is
                        # faster than the relu drain (~570ns/group)
# DVE:ACT relu split ~ 0.96:1.2 GHz incl. overheads -> 5:6 pattern
DVE_SLOTS = (0, 2, 4, 6, 8)
PERIOD = 11


def _build_m(filters: np.ndarray, areas: np.ndarray) -> np.ndarray:
    """M [128, 21384] fp8e4: placed-filter geometry + threshold rows + pad."""
    F = np.asarray(filters, dtype=np.float32).reshape(NF, 5, 5)
    M = np.zeros((KPAD, NF, NPOS), dtype=np.float32)
    for i in range(9):
        h = min(5, 9 - i)
        for j in range(9):
            w = min(5, 9 - j)
            blk = np.zeros((NF, 9, 9), dtype=np.float32)
            blk[:, i:i + h, j:j + w] = F[:, :h, :w]
            M[:NPOS, :, i * 9 + j] = blk.reshape(NF, NPOS).T
    ar = np.asarray(areas, dtype=np.float32).reshape(NF)
    thr = np.where(ar > 0.5, 1.0 - ar, -1.0)
    lo = np.floor(thr / 2.0)
    M[NPOS, :, :] = lo[:, None]          # floor(thr/2)   in [-12, 0]
    M[NPOS + 1, :, :] = (thr - lo)[:, None]  # ceil(thr/2) in [-12, 0]
    return M.reshape(KPAD, NCOL).astype(ml_dtypes.float8_e4m3)


def _build_boardt(board_free: np.ndarray) -> np.ndarray:
    """boardT [cores, 128, 512] fp8e4: transposed boards + ones rows + pad."""
    b = np.asarray(board_free, dtype=np.float32).reshape(N_CORES, BPC, NPOS)
    bt = np.zeros((N_CORES, KPAD, BPC), dtype=np.float32)
    bt[:, :NPOS, :] = b.transpose(0, 2, 1)
    bt[:, NPOS:K, :] = 1.0
    return bt.astype(ml_dtypes.float8_e4m3)


def _build_module():
    import concourse.bass as bass
    import concourse.mybir as mybir
    import concourse.tile as tile

    f32 = mybir.dt.float32
    fp8 = mybir.dt.float8e4
    Relu = mybir.ActivationFunctionType.Relu

    nc = bass.Bass("TRN2", target_bir_lowering=False, debug=False,
                   num_devices=N_CORES)

    boardt_d = nc.dram_tensor("boardt", [KPAD, BPC], fp8, kind="ExternalInput")
    m_d = nc.dram_tensor("mmat", [KPAD, NCOL], fp8, kind="ExternalInput")
    out_d = nc.dram_tensor("out", [BPC, NCOL], fp8, kind="ExternalOutput")

    with tile.TileContext(nc) as tc:
        with tc.tile_pool(name="const", bufs=1) as cpool:
            boardT = cpool.tile([KPAD, BPC], fp8)
            msb = cpool.tile([KPAD, NCOL], fp8)

            # Inputs on the ACT hwdge ring (board first), outputs on the SP
            # ring -- rings are FIFO per issuing engine, so output stores
            # never queue behind the M upload.  M streams in as 8 column
            # slabs; matmuls over a column range depend only on its slabs.
            nc.scalar.dma_start(boardT[:], boardt_d[:])
            slab = NCOL // N_SLABS
            for s in range(N_SLABS):
                nc.scalar.dma_start(msb[:, s * slab:(s + 1) * slab],
                                    m_d[:, s * slab:(s + 1) * slab])

            # ---- pipeline: PSUM ring (3 slots) + HAM bank + staging ----
            with (
                tc.tile_pool(name="wprep", bufs=1) as wprep,
                tc.tile_pool(name="psW", bufs=1, space="PSUM") as psW,
                tc.tile_pool(name="psM", bufs=3, space="PSUM") as psM,
                tc.tile_pool(name="ostage", bufs=4) as ostage,
            ):
                # HAM warm-up: fp32 dummy matmuls while inputs upload.  The
                # PE clock gate sits at 1.2 GHz until the activity monitor
                # sees ~3.4us of sustained matmul activity; these bridge
                # into the main loop so it starts at 2.4 GHz.
                dummy = wprep.tile([128, 256], f32, tag="wsrc")
                nc.vector.memset(dummy[:], 0.0)
                wps = psW.tile([128, 512], f32, tag="warm")
                for _ in range(WARMUP_MM):
                    nc.tensor.matmul(wps[:, 0:256], dummy[:, 0:128],
                                     dummy[:], start=True, stop=True)

                grp = 0
                for kb in range(BPC // 128):
                    lhsT = boardT[0:K, kb * 128:(kb + 1) * 128]
                    for g0 in range(0, NCOL, DMA_GRP):
                        dw = min(DMA_GRP, NCOL - g0)
                        ot = ostage.tile([128, DMA_GRP], fp8, tag="ot")
                        for h0 in range(0, dw, GRP):
                            hw = min(GRP, dw - h0)
                            pt = psM.tile([128, GRP], f32, tag="mm")
                            for q in range(0, hw, COL_TILE):
                                w = min(COL_TILE, hw - q)
                                c = g0 + h0 + q
                                nc.tensor.matmul(pt[:, q:q + w], lhsT,
                                                 msb[0:K, c:c + w],
                                                 start=True, stop=True)
                            # keep PE activity contiguous (see FILLER_MM)
                            for f in range(FILLER_MM):
                                nc.tensor.matmul(
                                    wps[:, 256 + f * 128:384 + f * 128],
                                    lhsT, msb[0:K, 0:128],
                                    start=True, stop=True)
                            if grp % PERIOD in DVE_SLOTS:
                                nc.vector.tensor_scalar_max(
                                    ot[:, h0:h0 + hw], pt[:, :hw], 0.0)
                            else:
                                nc.scalar.activation(ot[:, h0:h0 + hw],
                                                     pt[:, :hw], Relu)
                            grp += 1
                        nc.sync.dma_start(
                            out_d[kb * 128:(kb + 1) * 128, g0:g0 + dw],
                            ot[:, :dw])
                wrd = wprep.tile([32, 1], f32, tag="wrd")
                nc.vector.tensor_scalar_add(wrd[:], wps[0:32, 0:1], 0.0)
    return nc


def _legalize_multiwait(nc):
    """Split multi-wait instructions for this walrus build.

    The TPB instruction encodings carry exactly one semaphore wait, and
    the walrus codegen here refuses instructions with more ("Too many
    sync wait commands").  Hoist all but one wait onto EventSemaphore
    carrier instructions placed immediately before, on the same engine —
    the sequencer blocks on each carrier first, which is semantically
    identical.
    """
    import concourse.mybir as mybir

    for func in nc.m.functions:
        for blk in func.blocks:
            out = []
            changed = False
            for inst in blk.instructions:
                si = inst.sync_info
                waits = list(si.on_wait) if si is not None and si.on_wait else []
                if len(waits) > 1:
                    for j, w in enumerate(waits[:-1]):
                        carrier = mybir.InstEventSemaphore(
                            name=f"{inst.name}-xw{j}",
                            engine=inst.engine,
                            ins=[], outs=[],
                            sync_info=mybir.SyncInfo(on_wait=[w],
                                                     on_update=[]),
                        )
                        nc.register_instruction(carrier)
                        out.append(carrier)
                    inst.sync_info = mybir.SyncInfo(
                        on_wait=[waits[-1]],
                        on_update=list(si.on_update) if si.on_update else [])
                    changed = True
                out.append(inst)
            if changed:
                blk.instructions = out


_MODULE = None


def _get_module():
    global _MODULE
    if _MODULE is None:
        _MODULE = _build_module()
        _legalize_multiwait(_MODULE)
    return _MODULE


def run(board_free, filters, areas, trace=False, **spmd_kwargs):
    from concourse.bass_utils import run_bass_kernel_spmd

    boardt = _build_boardt(board_free)
    mmat = _build_m(filters, areas)

    in_maps = [
        {"boardt": boardt[c], "mmat": mmat}
        for c in range(N_CORES)
    ]
    nc = _get_module()
    res = run_bass_kernel_spmd(nc, in_maps, core_ids=list(range(N_CORES)),
                               trace=trace, **spmd_kwargs)
    out = np.concatenate(
        [np.asarray(r["out"]).astype(np.float32) for r in res.results], axis=0)
    out = out.reshape(BATCH, NF, 9, 9)
    return out, res


def kernel(board_free, filters, areas):
    out, _ = run(board_free, filters, areas)
    return out
